# revision 2
# baseline (speedup 1.0000x reference)
"""Deformable 1D convolution for Trainium2 (8 NeuronCores, data-parallel over batch).

Math (validated against the reference):
    p[t,k]   = clip(k + offsets[b,0,t,k], 0, 2)
    c[k,j,t] = mask[b,k,t] * relu(1 - |p[t,k] - j|)      j in {0,1,2}
    out[b,o,t] = sum_{k,j} c[k,j,t] * (W_k @ x[b])[o, t+j] + bias[o]

Kernel strategy (v6 "banded"): move the per-t coefficient application onto
the PE as a banded-matrix contraction, eliminating the elementwise 9-term
bottleneck of v5.

  Per chunk of CH=125 output positions (s-range 127 = t+j):
    MM1:    Z[s, (k,o)] = x[:, t0:t0+127]^T @ Wr          (PSUM, 384 cols)
    copy:   Zb = bf16(Z)  split Act (288 cols) / DVE (96 cols)
    MM2-4:  ps[o, t] += Zb[:, kC:(k+1)C]^T @ A_k[s, t]    (PSUM accum over k)
            A_k[s, t] = c[k, s-t, t0+t] — banded coefficient matrix built on
            the HOST (pure function of offsets/mask) and DMAed in bf16.
    every 4 chunks: DVE tensor_scalar adds bias (per-partition: out is [o, t])
            and casts PSUM -> bf16 SBUF; DMA out every 8 chunks.

  Output is produced directly in [C_out, t] layout => host unshard is a
  concat, no transpose. PE issue order is software-pipelined (MM1 of chunk
  n+1 issues before MM2-4 of chunk n) to hide the Zb copy latency.
"""

import numpy as np
import ml_dtypes
from contextlib import ExitStack

import concourse.bass as bass
import concourse.mybir as mybir
import concourse.tile as tile
from concourse import bacc
from concourse import bass_utils

F32 = mybir.dt.float32
BF16 = mybir.dt.bfloat16
OP = mybir.AluOpType
ACTF = mybir.ActivationFunctionType

B, C, L, K = 16, 128, 4096, 3
LOUT = L - (K - 1)          # 4094
NCORES = 8
BPC = B // NCORES           # batches per core (2)
CH = 125                    # output positions per chunk
NS = -(-LOUT // CH)         # 33 chunks
AW = 3 * 126                # A-tile cols per chunk (3 k-blocks of 126, 125 used)
PSG = 4                     # chunks per PSUM output group (4*125 = 500 <= 512)
OG = 8                      # chunks per output DMA group (8*125 = 1000 cols)

_CACHE = {}


def _chunk_dims(s):
    t0 = s * CH
    tc = min(CH, LOUT - t0)            # valid outputs in this chunk
    sr = min(127, L - t0)              # s-range (contraction size)
    return t0, tc, sr


def _build_program():
    if "nc" in _CACHE:
        return _CACHE["nc"]

    nc = bacc.Bacc(
        "TRN2",
        target_bir_lowering=False,
        debug=False,
        enable_asserts=False,
        num_devices=NCORES,
    )

    x_in = nc.dram_tensor("x_in", [BPC, C, L], BF16, kind="ExternalInput").ap()
    a_in = nc.dram_tensor("a_in", [BPC, 128, NS * AW], BF16,
                          kind="ExternalInput").ap()
    wt = nc.dram_tensor("wt", [C, K * C], BF16, kind="ExternalInput").ap()
    bvec = nc.dram_tensor("bvec", [128, 1], F32, kind="ExternalInput").ap()
    outT = nc.dram_tensor("outT", [BPC, C, LOUT], BF16, kind="ExternalOutput").ap()

    with tile.TileContext(nc) as tc, ExitStack() as ctx:
        const_pool = ctx.enter_context(tc.tile_pool(name="const", bufs=1))
        x_pool = ctx.enter_context(tc.tile_pool(name="x", bufs=2))
        a_pool = ctx.enter_context(tc.tile_pool(name="a", bufs=2))
        zb_pool = ctx.enter_context(tc.tile_pool(name="zb", bufs=4))
        o_pool = ctx.enter_context(tc.tile_pool(name="o", bufs=2))
        zps_pool = ctx.enter_context(tc.tile_pool(name="zps", bufs=4, space="PSUM"))
        ops_pool = ctx.enter_context(tc.tile_pool(name="ops", bufs=2, space="PSUM"))

        wt_sb = const_pool.tile([128, K * C], BF16)
        nc.sync.dma_start(wt_sb[:], wt[:])
        bias_sb = const_pool.tile([128, 1], F32)
        nc.sync.dma_start(bias_sb[:], bvec[:])

        for b in range(BPC):
            # x for this batch: 3 slices so chunk 0 starts early
            x_sb = x_pool.tile([128, L], BF16)
            for c0, c1 in ((0, 512), (512, 2048), (2048, L)):
                nc.sync.dma_start(x_sb[:, c0:c1], x_in[b][:, c0:c1])
            # banded coefficient tiles: 4 slices (chunk ranges)
            a_sb = a_pool.tile([128, NS * AW], BF16)
            for s0, s1 in ((0, 5), (5, 15), (15, 25), (25, NS)):
                nc.sync.dma_start(a_sb[:, s0 * AW:s1 * AW],
                                  a_in[b][:, s0 * AW:s1 * AW])

            # software pipeline: stage1[s] = MM1 + Zb copy; stage2[s] = MM2-4
            zb_tiles = {}

            def stage1(s):
                t0, tc_, sr = _chunk_dims(s)
                zps = zps_pool.tile([128, K * C], F32, tag="zps")
                nc.tensor.matmul(zps[0:sr, :], x_sb[:, t0:t0 + sr], wt_sb[:, :],
                                 start=True, stop=True)
                zb = zb_pool.tile([128, K * C], BF16, tag="zb")
                nc.scalar.activation(zb[0:sr, 0:288], zps[0:sr, 0:288], ACTF.Copy)
                nc.vector.tensor_copy(zb[0:sr, 288:384], zps[0:sr, 288:384])
                zb_tiles[s] = zb

            def stage2(s, ops):
                t0, tc_, sr = _chunk_dims(s)
                zb = zb_tiles.pop(s)
                gi = s % PSG
                for k in range(K):
                    nc.tensor.matmul(
                        ops[0:128, gi * CH:(gi + 1) * CH],
                        zb[0:sr, k * C:(k + 1) * C],
                        a_sb[0:sr, s * AW + k * 126:s * AW + k * 126 + CH],
                        start=(k == 0),
                        stop=(k == K - 1),
                    )

            out_sb = None
            ops = None
            stage1(0)
            for s in range(NS):
                if s + 1 < NS:
                    stage1(s + 1)
                if s % PSG == 0:
                    ops = ops_pool.tile([128, PSG * CH], F32, tag="ops")
                stage2(s, ops)
                if s % PSG == PSG - 1 or s == NS - 1:
                    # bias add + cast for the finished PSUM group
                    g0 = (s // PSG) * PSG
                    t0g = g0 * CH
                    ncols = min(PSG * CH, LOUT - t0g)
                    if s % OG < PSG:
                        out_sb = o_pool.tile([128, OG * CH], BF16, tag="osb")
                        off = 0
                    else:
                        off = PSG * CH
                    nc.vector.tensor_scalar(
                        out_sb[:, off:off + ncols], ops[0:128, 0:ncols],
                        bias_sb[:, 0:1], None, OP.add,
                    )
                    if s % OG == OG - 1 or s == NS - 1:
                        d0 = (s // OG) * OG * CH
                        dn = min(OG * CH, LOUT - d0)
                        nc.sync.dma_start(outT[b][:, d0:d0 + dn],
                                          out_sb[:, 0:dn])

    nc.compile()
    _CACHE["nc"] = nc
    return nc


def _make_in_maps(x, offsets, mask, weight, bias):
    x = np.asarray(x, dtype=np.float32)
    offsets = np.asarray(offsets, dtype=np.float32)
    mask = np.asarray(mask, dtype=np.float32)
    weight = np.asarray(weight, dtype=np.float32)
    bias = np.asarray(bias, dtype=np.float32)

    bf16 = ml_dtypes.bfloat16
    x_bf = np.ascontiguousarray(x.astype(bf16))
    # wt[c, k*C + o] = weight[o, c, k]
    wt = np.ascontiguousarray(
        weight.transpose(1, 2, 0).reshape(C, K * C).astype(bf16)
    )
    bvec = np.ascontiguousarray(bias.reshape(128, 1))

    # coefficients c[b, k, j, t] = mask * relu(1 - |clip(k + off, 0, 2) - j|)
    off = offsets[:, 0]                                   # [B, LOUT, K]
    p = np.clip(np.arange(K, dtype=np.float32) + off, 0.0, 2.0)  # [B, T, K]
    j = np.arange(3, dtype=np.float32).reshape(1, 1, 1, 3)
    u = np.maximum(0.0, 1.0 - np.abs(p[..., None] - j))   # [B, T, K, 3]
    cf = u * mask.transpose(0, 2, 1)[..., None]           # [B, T, K, 3]

    # pad t to NS*CH, reshape into chunks
    LP = NS * CH
    cfp = np.zeros((B, LP, K, 3), np.float32)
    cfp[:, :LOUT] = cf
    cfc = cfp.reshape(B, NS, CH, K, 3).astype(bf16)       # [B, NS, tl, K, j]

    # banded tiles A[b, ns, s(128), k, 126]: A[.., tl+j, k, tl] = cfc[.., tl, k, j]
    A = np.zeros((B, NS, 128, K, 126), bf16)
    tl = np.arange(CH)
    for jj in range(3):
        # advanced indexing puts the tl axis first on both sides
        A[:, :, tl + jj, :, tl] = np.moveaxis(cfc[:, :, :, :, jj], 2, 0)

    in_maps = []
    for cid in range(NCORES):
        sl = slice(cid * BPC, (cid + 1) * BPC)
        a_core = A[sl].transpose(0, 2, 1, 3, 4).reshape(BPC, 128, NS * AW)
        in_maps.append({
            "x_in": np.ascontiguousarray(x_bf[sl]),
            "a_in": np.ascontiguousarray(a_core),
            "wt": wt,
            "bvec": bvec,
        })
    return in_maps


def kernel(x, offsets, mask, weight, bias):
    nc = _build_program()
    in_maps = _make_in_maps(x, offsets, mask, weight, bias)
    res = bass_utils.run_bass_kernel_spmd(nc, in_maps, core_ids=list(range(NCORES)))
    out = np.empty((B, C, LOUT), np.float32)
    for cid in range(NCORES):
        out[cid * BPC:(cid + 1) * BPC] = res.results[cid]["outT"].astype(np.float32)
    return out


# revision 6
# speedup vs baseline: 1.0804x; 1.0804x over previous
"""Deformable 1D convolution for Trainium2 (8 NeuronCores, data-parallel over batch).

Math (validated against the reference):
    p[t,k]   = clip(k + offsets[b,0,t,k], 0, 2)
    c[k,j,t] = mask[b,k,t] * relu(1 - |p[t,k] - j|)      j in {0,1,2}
    out[b,o,t] = sum_{k,j} c[k,j,t] * (W_k @ x[b])[o, t+j] + bias[o]

v8 "banded-first": both heavy stages run on the PE.

  Host prepares, per chunk i of CH=126 outputs:
    xT_stag[:, 128i:128(i+1)] = x^T rows 126i..126i+128   (staggered x^T)
    A[p, k*126+t] = c[k, p-t, 126i+t] for p-t in {0,1,2}  (banded coeffs, bf16)

  Per chunk (one 128-contraction matmul each):
    MM-B: xs[c, (k,t)] = xT_blk^T @ A        (378 cols; one stationary for all k)
    copy: xs -> bf16 SBUF, split Act/DVE
  Per group of G=4 chunks (stationary W_k amortized over the group):
    MM-W k=0..2: out'[o, (g,t)] += W_k^T @ xs_k[c, (g,t)]  (strided rhs AP)
    Pool: bias add (per-partition, out' is [C_out, t]) + bf16 cast
  DMA out every 2 groups. Output layout [C_out, t] => host unshard is a concat.
"""

import numpy as np
import ml_dtypes
from contextlib import ExitStack

import concourse.bass as bass
import concourse.mybir as mybir
import concourse.tile as tile
from concourse import bacc
from concourse import bass_utils
from concourse.ap import AP

F32 = mybir.dt.float32
BF16 = mybir.dt.bfloat16
OP = mybir.AluOpType
ACTF = mybir.ActivationFunctionType

B, C, L, K = 16, 128, 4096, 3
LOUT = L - (K - 1)          # 4094
NCORES = 8
BPC = B // NCORES           # batches per core (2)
CH = 126                    # output positions per chunk
NS = -(-LOUT // CH)         # 33 chunks
AW = K * CH                 # A-tile cols per chunk (378)
XW = NS * 128               # staggered xT cols per batch (4224)
G = 4                       # chunks per W-group / PSUM output bank
NG = -(-NS // G)            # 9 groups (8 full + 1 single-chunk)
OG = 2 * G                  # chunks per output DMA (8*126 = 1008 cols)

_CACHE = {}


def _build_program():
    if "nc" in _CACHE:
        return _CACHE["nc"]

    nc = bacc.Bacc(
        "TRN2",
        target_bir_lowering=False,
        debug=False,
        enable_asserts=False,
        num_devices=NCORES,
    )

    xt_in = nc.dram_tensor("xt_in", [BPC, 128, XW], BF16, kind="ExternalInput").ap()
    a_in = nc.dram_tensor("a_in", [BPC, 128, NS * AW], BF16,
                          kind="ExternalInput").ap()
    wt = nc.dram_tensor("wt", [C, K * C], BF16, kind="ExternalInput").ap()
    bvec = nc.dram_tensor("bvec", [128, 1], F32, kind="ExternalInput").ap()
    outT = nc.dram_tensor("outT", [BPC, C, LOUT], BF16, kind="ExternalOutput").ap()

    with tile.TileContext(nc) as tc, ExitStack() as ctx:
        const_pool = ctx.enter_context(tc.tile_pool(name="const", bufs=1))
        xt_pool = ctx.enter_context(tc.tile_pool(name="xt", bufs=2))
        a_pool = ctx.enter_context(tc.tile_pool(name="a", bufs=2))
        xs_pool = ctx.enter_context(tc.tile_pool(name="xs", bufs=2))
        o_pool = ctx.enter_context(tc.tile_pool(name="o", bufs=2))
        xps_pool = ctx.enter_context(tc.tile_pool(name="xps", bufs=6, space="PSUM"))
        ops_pool = ctx.enter_context(tc.tile_pool(name="ops", bufs=2, space="PSUM"))

        wt_sb = const_pool.tile([128, K * C], BF16)
        nc.sync.dma_start(wt_sb[:], wt[:])
        bias_sb = const_pool.tile([128, 1], F32)
        nc.sync.dma_start(bias_sb[:], bvec[:])

        for b in range(BPC):
            xt_sb = xt_pool.tile([128, XW], BF16)
            for c0, c1 in ((0, 1024), (1024, 2560), (2560, XW)):
                nc.sync.dma_start(xt_sb[:, c0:c1], xt_in[b][:, c0:c1])
            a_sb = a_pool.tile([128, NS * AW], BF16)
            for s0, s1 in ((0, 5), (5, 15), (15, 25), (25, NS)):
                nc.sync.dma_start(a_sb[:, s0 * AW:s1 * AW],
                                  a_in[b][:, s0 * AW:s1 * AW])

            # per-group state
            xs_tiles = {}   # group -> xs_sb tile [128, G*AW]

            def stage_b(i):
                """MM-B + PSUM->SBUF copy for chunk i."""
                g, gi = divmod(i, G)
                xps = xps_pool.tile([128, AW], F32, name="xps", tag="xps")
                nc.tensor.matmul(xps[0:128, :], xt_sb[:, 128 * i:128 * (i + 1)],
                                 a_sb[:, i * AW:(i + 1) * AW],
                                 start=True, stop=True)
                if g not in xs_tiles:
                    xs_tiles[g] = xs_pool.tile([128, G * AW], BF16, name="xs", tag="xs")
                xs = xs_tiles[g]
                o0 = gi * AW
                nc.scalar.activation(xs[:, o0:o0 + 192], xps[:, 0:192], ACTF.Copy)
                nc.vector.tensor_copy(xs[:, o0 + 192:o0 + AW], xps[:, 192:AW])

            def stage_w(g, out_sb):
                """W-contraction + bias/cast for group g."""
                gn = min(G, NS - g * G)          # chunks in this group
                t0g = g * G * CH
                ncols = min(gn * CH, LOUT - t0g)  # valid out cols
                xs = xs_tiles.pop(g)
                ops = ops_pool.tile([128, G * CH], F32, name="ops", tag="ops")
                for k in range(K):
                    rhs = AP(xs.tensor, k * CH,
                             [[G * AW, 128], [AW, gn], [1, CH]])
                    nc.tensor.matmul(ops[0:128, 0:gn * CH],
                                     wt_sb[:, k * C:(k + 1) * C], rhs,
                                     start=(k == 0), stop=(k == K - 1))
                off = (g % 2) * G * CH
                nc.vector.tensor_scalar(out_sb[:, off:off + ncols],
                                        ops[0:128, 0:ncols],
                                        bias_sb[:, 0:1], None, OP.add)

            out_sb = None
            LOOKAHEAD = 2
            for i in range(LOOKAHEAD):
                stage_b(i)
            for g in range(NG):
                if g % 2 == 0:
                    out_sb = o_pool.tile([128, OG * CH], BF16, name="osb", tag="osb")
                # emit MM-B for chunks of the NEXT groups (lookahead)
                for i in range(g * G + LOOKAHEAD, min((g + 1) * G + LOOKAHEAD, NS)):
                    stage_b(i)
                stage_w(g, out_sb)
                if g % 2 == 1 or g == NG - 1:
                    d0 = (g // 2) * OG * CH
                    dn = min(OG * CH, LOUT - d0)
                    nc.sync.dma_start(outT[b][:, d0:d0 + dn], out_sb[:, 0:dn])

    nc.compile()
    _CACHE["nc"] = nc
    return nc


def _make_in_maps(x, offsets, mask, weight, bias):
    x = np.asarray(x, dtype=np.float32)
    offsets = np.asarray(offsets, dtype=np.float32)
    mask = np.asarray(mask, dtype=np.float32)
    weight = np.asarray(weight, dtype=np.float32)
    bias = np.asarray(bias, dtype=np.float32)

    bf16 = ml_dtypes.bfloat16
    # staggered xT: xts[b, p, 128i + c] = x[b, c, 126i + p]
    xts = np.zeros((B, 128, XW), bf16)
    x_bf = x.astype(bf16)
    for i in range(NS):
        t0 = CH * i
        n = min(128, L - t0)
        xts[:, :n, 128 * i:128 * i + C] = x_bf[:, :, t0:t0 + n].transpose(0, 2, 1)
    # wt[c, k*C + o] = weight[o, c, k]
    wt = np.ascontiguousarray(
        weight.transpose(1, 2, 0).reshape(C, K * C).astype(bf16)
    )
    bvec = np.ascontiguousarray(bias.reshape(128, 1))

    # coefficients c[b, t, k, j] = mask * relu(1 - |clip(k + off, 0, 2) - j|)
    off = offsets[:, 0]                                   # [B, LOUT, K]
    p = np.clip(np.arange(K, dtype=np.float32) + off, 0.0, 2.0)
    j = np.arange(3, dtype=np.float32).reshape(1, 1, 1, 3)
    u = np.maximum(0.0, 1.0 - np.abs(p[..., None] - j))   # [B, T, K, 3]
    cf = u * mask.transpose(0, 2, 1)[..., None]           # [B, T, K, 3]

    LP = NS * CH
    cfp = np.zeros((B, LP, K, 3), np.float32)
    cfp[:, :LOUT] = cf
    cfc = cfp.reshape(B, NS, CH, K, 3).astype(bf16)       # [B, NS, tl, K, j]

    # banded tiles A[b, ns, p(128), k, 126]: A[.., tl+j, k, tl] = cfc[.., tl, k, j]
    A = np.zeros((B, NS, 128, K, CH), bf16)
    tl = np.arange(CH)
    for jj in range(3):
        A[:, :, tl + jj, :, tl] = np.moveaxis(cfc[:, :, :, :, jj], 2, 0)

    in_maps = []
    for cid in range(NCORES):
        sl = slice(cid * BPC, (cid + 1) * BPC)
        a_core = A[sl].transpose(0, 2, 1, 3, 4).reshape(BPC, 128, NS * AW)
        in_maps.append({
            "xt_in": np.ascontiguousarray(xts[sl]),
            "a_in": np.ascontiguousarray(a_core),
            "wt": wt,
            "bvec": bvec,
        })
    return in_maps


def kernel(x, offsets, mask, weight, bias):
    nc = _build_program()
    in_maps = _make_in_maps(x, offsets, mask, weight, bias)
    res = bass_utils.run_bass_kernel_spmd(nc, in_maps, core_ids=list(range(NCORES)))
    out = np.empty((B, C, LOUT), np.float32)
    for cid in range(NCORES):
        out[cid * BPC:(cid + 1) * BPC] = res.results[cid]["outT"].astype(np.float32)
    return out
